# revision 75
# baseline (speedup 1.0000x reference)
"""Causal multi-head attention block (B=2, T=2048, C=1024, H=16) on 8 TRN2
NeuronCores.

Sharding: 2D tensor parallel — core r owns batch b = r//4 and head quad
g = r%4 (heads 4g..4g+3, feature slice [256g, 256g+256)). Each core
projects q/k/v for its 256 features over its batch's 2048 tokens
(x^T replicated per batch) and runs causal attention for its 4 heads.
Per 512-token block, the 4 cores of a batch AllGather their y shards
(~0.75 MB wire/core per block at 40-60 GB/s — measured ~1.7x faster
than the ReduceScatter-of-partial-outputs dual, which drags through the
CCE reduction path), then each core computes its own 256 output rows
with a full-C contraction and writes them straight to `out`.

On-device everything is feature-major (transposed) so the TensorEngine
contraction axis sits on SBUF partitions and the softmax denominator
arrives via a ones-column appended to V:

  qT/kT [128, 2, 2048] = W_shard @ x^T           (2 feature tiles)
  v_sb[128tok, kt, h, 0:64] = x_tile^T @ Wv      (v built NATURALLY per
                                                  token tile — no
                                                  transpose pass)
  ST tile [128k, 512q] = kT[64h slice].T @ qT    (contract d=64)
  PT = exp(ST * 1/sqrt(d))                       (no max-subtraction:
                                                  logits ~N(0,1))
  causal mask: PT[diag 128-block] *= tri (upper-triangular 0/1 bf16,
      vector multiply) — cheaper than injecting -1e9 into PSUM via
      matmul, which cost an ident LDWEIGHTS + matmul per diagonal tile
  yT [65, 512] += [v | 1].T @ PT                 (row 64 = softmax denom)
  ych[*, 512]  = yT[0:64] * bcast(recip(yT[64]))
  out[128rt]  += WoT[ci, rt] @ ytf_gathered[ci]  (2 row tiles × 8 ci)

Performance structure (from perfetto/HAM analysis): the PE idle-activity
throttle (HAM) halves the clock for any window containing idle time — a
dense-matmul microbench holds 2.4 GHz while the v1 kernel averaged
~0.63x. So the whole kernel is ONE software-pipelined job stream:
score-pair jobs and filler GEMM jobs (projection chunks, O-projection
groups, v tiles) are interleaved, with each pair's PV matmuls deferred
LA jobs so the PE never waits on the exp of the pair it just scored,
and each head's normalization deferred further so neither the PE nor
gpsimd ever head-of-line blocks on the vector eviction chain (a gpsimd
<-> vector cross-wait stalled every engine for ~20 us). The last
512-token block is split into two 256-wide halves so the final
AllGather is half-size and the previous one overlaps attention.

Inputs are bf16 (host-side cast); accumulation is f32 in PSUM; the
output shard is written bf16 and upcast to f32 on the host.
"""

import numpy as np
import ml_dtypes

import concourse.bacc as bacc
import concourse.mybir as mybir
import concourse.tile as tile
from concourse.bass_utils import run_bass_kernel_spmd
from concourse.masks import make_identity

N_CORES = 8
B, T, C, H = 2, 2048, 1024, 16
D = 64                # head dim
GR = 4                # head-group cores per batch
HL = 4                # heads per core
DL = HL * D           # local feature dim = 256
MT = DL // 128        # feature tiles per core = 2
TL = T                # local tokens per core = its batch's 2048
P = 128
NCH = C // P          # 8 contraction chunks for q/k/v projections
QCH = 512             # q-chunk / token chunk
NQC = TL // QCH       # 4 chunks
NKT = TL // P         # 16 k-tiles
NRT = C // P          # 8 output row tiles for partial O-proj
SCALE = 1.0 / np.sqrt(D)
LA = 2                # job-stream lookahead (score pairs ahead of PV)

BF = mybir.dt.bfloat16
F32 = mybir.dt.float32
AF = mybir.ActivationFunctionType

RGROUPS = [[0, 1, 2, 3], [4, 5, 6, 7]]


def build_graph():
    nc = bacc.Bacc("TRN2", target_bir_lowering=False, debug=False)

    xT = nc.dram_tensor("xT", [C, TL], BF, kind="ExternalInput")
    # qkv shards [p, w, ci, m(256)] then woT [p, ci2, rt*128]
    WQKV = 3 * NCH * DL
    WO = MT * C
    wall = nc.dram_tensor("wall", [P, WQKV + WO], BF, kind="ExternalInput")
    out = nc.dram_tensor("out", [DL, TL], BF, kind="ExternalOutput")

    with tile.TileContext(nc) as tc:
        with (
            tc.tile_pool(name="sb", bufs=1) as sb,
            tc.tile_pool(name="ps", bufs=1, space="PSUM") as ps,
            tc.tile_pool(name="dram", bufs=1, space="DRAM") as dram,
        ):
            # ---- loads ----
            # separate tiles per weight and per x-chunk group so a
            # consumer only waits on ITS producer DMA, not all of them
            wq_full = sb.tile([P, NCH * DL], BF, name="wq_full")
            wk_full = sb.tile([P, NCH * DL], BF, name="wk_full")
            wv_full = sb.tile([P, NCH * DL], BF, name="wv_full")
            wo_full = sb.tile([P, MT * C], BF, name="wo_full")
            # x split: first 512 tokens per ci land fast, rest batched
            x_head = sb.tile([P, NCH, QCH], BF, name="x_head")
            x_tail = sb.tile([P, NCH, TL - QCH], BF, name="x_tail")

            # ident + wsrc FIRST: the warmup matmuls gate on them, and
            # make_identity runs on gpsimd — queued behind the DMA
            # issues it delayed the very first matmul to ~12us
            ident = sb.tile([P, P], BF, name="ident")
            make_identity(nc, ident)
            wsrc = sb.tile([P, QCH], BF, name="wsrc")
            nc.vector.memset(wsrc[:], 0.5)

            def keepwarm(n):
                for _ in range(n):
                    wdst = ps.tile([P, QCH], F32, tag="st", bufs=3,
                                   name="wdst")
                    nc.tensor.matmul(wdst[:], ident[:], wsrc[:],
                                     start=True, stop=True)

            keepwarm(24)

            ENGS3 = (nc.scalar, nc.sync, nc.gpsimd)
            # weights are packed mt-major host-side so each mt half is a
            # contiguous 256 KB — the first projection job then gates on
            # a half-DMA instead of the whole weight
            WH = NCH * DL // 2
            nc.sync.dma_start(wq_full[:, 0:WH], wall[:, 0:WH])
            for ci in (1, 4):
                ENGS3[ci % 3].dma_start(
                    x_head[:, ci, :], xT[ci * P:(ci + 1) * P, 0:QCH]
                )
            nc.sync.dma_start(wq_full[:, WH:], wall[:, WH:2 * WH])
            for ci in (0, 3, 6, 2, 5):
                ENGS3[ci % 3].dma_start(
                    x_head[:, ci, :], xT[ci * P:(ci + 1) * P, 0:QCH]
                )
            nc.scalar.dma_start(wk_full[:, 0:WH], wall[:, 2 * WH:3 * WH])
            nc.scalar.dma_start(wk_full[:, WH:], wall[:, 3 * WH:4 * WH])
            nc.gpsimd.dma_start(wv_full[:],
                                wall[:, 2 * NCH * DL:3 * NCH * DL])
            nc.sync.dma_start(x_head[:, 7, :], xT[7 * P:8 * P, 0:QCH])
            for ci in range(NCH):
                ENGS3[ci % 3].dma_start(
                    x_tail[:, ci, :], xT[ci * P:(ci + 1) * P, QCH:TL]
                )
            nc.scalar.dma_start(wo_full[:], wall[:, WQKV:])

            def xap(ci, t0, t1):
                # token-range view across the head/tail split tiles
                assert t0 >= QCH or t1 <= QCH
                if t1 <= QCH:
                    return x_head[:, ci, t0:t1]
                return x_tail[:, ci, t0 - QCH:t1 - QCH]

            # mt-major: [p, mt, ci, 128]
            wq_sb = wq_full[:].rearrange("p (m a c) -> p m a c", m=MT, a=NCH)
            wk_sb = wk_full[:].rearrange("p (m a c) -> p m a c", m=MT, a=NCH)
            wv_sb = wv_full[:].rearrange("p (m a c) -> p m a c", m=MT, a=NCH)
            wo_sb = wo_full[:].rearrange("p (a r) -> p a r", a=NCH)

            # upper-triangular (q >= k) 0/1 mask for diagonal blocks
            tri = sb.tile([P, P], BF, name="tri")
            nc.gpsimd.memset(tri[:], 1.0)
            nc.gpsimd.affine_select(
                out=tri[:], in_=tri[:],
                compare_op=mybir.AluOpType.is_ge,
                fill=0.0, base=0, channel_multiplier=-1, pattern=[[1, P]],
            )

            qT_sb = sb.tile([P, MT, TL], BF, name="qT_sb")
            kT_sb = sb.tile([P, MT, TL], BF, name="kT_sb")
            # v natural layout, per 128-token tile per head: [64 v | 1]
            v_sb = sb.tile([P, NKT, HL, D + 1], BF, name="v_sb")
            nc.gpsimd.memset(v_sb[:], 1.0)

            # blocks: (q0, qw, nkt). The last 512-token block is split
            # into two 256-wide halves so its first RS overlaps the
            # remaining attention and the tail RS is half-size.
            BLOCKS = [
                (0, QCH, 4), (QCH, QCH, 8), (2 * QCH, QCH, 12),
                (3 * QCH, QCH // 2, 14), (3 * QCH + QCH // 2, QCH // 2, 16),
            ]
            NBLK = len(BLOCKS)

            # the LAST block's gather is split per feature half so the
            # first half flies while heads 2-3 are still normalizing
            ag_in = [
                dram.tile([DL, qw], BF, name=f"ag_in{c}")
                for c, (q0, qw, nkt) in enumerate(BLOCKS[:-1])
            ]
            ytf = [
                dram.tile([C, qw], BF, name=f"ytf{c}")
                for c, (q0, qw, nkt) in enumerate(BLOCKS[:-1])
            ]
            QWL = BLOCKS[-1][1]
            ag_last = [dram.tile([P, QWL], BF, name=f"ag_last{m}")
                       for m in range(MT)]
            ytf_last = [dram.tile([GR * P, QWL], BF, name=f"ytf_last{m}")
                        for m in range(MT)]

            # ---------- job bodies ----------
            def proj_one(tch, wsb, mt, dst):
                # single 512-wide chunk (used early: evicts ASAP)
                t0 = tch * QCH
                pj = ps.tile([P, QCH], F32, tag="st", bufs=3, name="pj")
                for ci in range(NCH):
                    nc.tensor.matmul(
                        pj[:], wsb[:, mt, ci, :],
                        xap(ci, t0, t0 + QCH),
                        start=(ci == 0), stop=(ci == NCH - 1),
                    )
                nc.vector.tensor_copy(dst[:, mt, t0:t0 + QCH], pj[:])

            def proj_pair(pch, wsb, mt, dst):
                # two 512-wide accumulation groups, one 1024-wide evict
                t0 = pch * 2 * QCH
                pj = ps.tile([P, 2 * QCH], F32, tag="st", bufs=3, name="pj")
                for half in range(2):
                    h0 = t0 + half * QCH
                    for ci in range(NCH):
                        nc.tensor.matmul(
                            pj[:, half * QCH:(half + 1) * QCH],
                            wsb[:, mt, ci, :],
                            xap(ci, h0, h0 + QCH),
                            start=(ci == 0), stop=(ci == NCH - 1),
                        )
                nc.vector.tensor_copy(
                    dst[:, mt, t0:t0 + 2 * QCH], pj[:]
                )

            def v_tile(t16):
                # v_nat [128 tok, 256 feat] = x_tile^T @ Wv
                vps = ps.tile([P, DL], F32, tag="st", bufs=3, name="vps")
                for ci in range(NCH):
                    nc.tensor.matmul(
                        vps[:], xap(ci, t16 * P, (t16 + 1) * P),
                        wv_sb[:, :, ci, :],
                        start=(ci == 0), stop=(ci == NCH - 1),
                    )
                nc.vector.tensor_copy(
                    v_sb[:, t16, :, 0:D],
                    vps[:].rearrange("p (h x) -> p h x", h=HL),
                )

            ych_tiles = {}
            yt_tiles = {}

            def scores(blk, h, pr):
                """Scores + exp (+ causal mask) for one k-tile pair."""
                q0, qw, nkt = BLOCKS[blk]
                mt, hh = h // 2, h % 2
                rsl = slice(hh * D, (hh + 1) * D)
                st = ps.tile([P, 2 * qw], F32, tag="st", bufs=3, name="st")
                pt = sb.tile([P, 2 * qw], BF, tag="pt", bufs=5, name="pt")
                diag = []
                for half in range(2):
                    kt = 2 * pr + half
                    qv = max(kt * P - q0, 0)
                    nc.tensor.matmul(
                        st[:, half * qw + qv:(half + 1) * qw],
                        kT_sb[rsl, mt, kt * P:(kt + 1) * P],
                        qT_sb[rsl, mt, q0 + qv:q0 + qw],
                        start=True, stop=True,
                    )
                    if kt * P >= q0:
                        diag.append(half * qw + qv)
                qv0 = max(2 * pr * P - q0, 0)
                nc.scalar.activation(
                    pt[:, qv0:], st[:, qv0:], AF.Exp, scale=float(SCALE)
                )
                for c0 in diag:
                    nc.vector.tensor_mul(
                        pt[:, c0:c0 + P], pt[:, c0:c0 + P], tri[:]
                    )
                return pt

            def pv(blk, h, pr, pt):
                q0, qw, nkt = BLOCKS[blk]
                yt = yt_tiles.get((blk, h))
                if yt is None:
                    yt = yt_tiles[(blk, h)] = ps.tile(
                        [D + 1, qw], F32, tag="yt", bufs=2, name="yt"
                    )
                for half in range(2):
                    kt = 2 * pr + half
                    qv = max(kt * P - q0, 0)
                    nc.tensor.matmul(
                        yt[:, qv:qw],
                        v_sb[:, kt, h, :],
                        pt[:, half * qw + qv:(half + 1) * qw],
                        start=(kt == 0), stop=(kt == nkt - 1),
                    )

            den_tiles = {}

            def stage_den(blk, h):
                # with the last pv: denominator row PSUM -> SBUF bf16
                q0, qw, nkt = BLOCKS[blk]
                yt = yt_tiles[(blk, h)]
                den = sb.tile([1, qw], F32, tag="den", bufs=4, name="den")
                nc.vector.tensor_copy(den[:], yt[D:D + 1, :])
                den_tiles[(blk, h)] = den

            def norm(blk, h):
                """Scheduled >=LA+1 jobs after the head's last pair so
                the gpsimd broadcast never head-of-line blocks against
                the vector den copy (that cross-wait stalled ~20us)."""
                q0, qw, nkt = BLOCKS[blk]
                yt = yt_tiles.pop((blk, h))
                den = den_tiles.pop((blk, h))
                bc = sb.tile([D, qw], F32, tag="bc", bufs=3, name="bc")
                nc.gpsimd.partition_broadcast(bc[:], den[:])
                rcp = sb.tile([D, qw], F32, tag="rcp", bufs=3, name="rcp")
                scr = sb.tile([D, qw], F32, tag="scr", bufs=3, name="scr")
                nc.vector.reciprocal_approx_accurate(
                    rcp[:], bc[:], scratch=scr[:]
                )
                mt, hh = h // 2, h % 2
                ych = ych_tiles[blk]
                nc.vector.tensor_mul(
                    ych[hh * D:(hh + 1) * D, mt, :], yt[0:D, :], rcp[:]
                )
                if blk == NBLK - 1:
                    # tail block: gather each feature half independently
                    # (NOT per head: sub-128KB collectives pay ~5-10us
                    # of per-op stream overhead and serialize the tail)
                    if h == 1:
                        nc.sync.dma_start(ag_last[0][:], ych[:, 0, :])
                        nc.gpsimd.collective_compute(
                            "AllGather", mybir.AluOpType.bypass,
                            replica_groups=RGROUPS,
                            ins=[ag_last[0][:]], outs=[ytf_last[0][:]],
                        )
                    elif h == HL - 1:
                        ych = ych_tiles.pop(blk)
                        nc.sync.dma_start(ag_last[1][:], ych[:, 1, :])
                        nc.gpsimd.collective_compute(
                            "AllGather", mybir.AluOpType.bypass,
                            replica_groups=RGROUPS,
                            ins=[ag_last[1][:]], outs=[ytf_last[1][:]],
                        )
                    return
                aview = ag_in[blk][:].rearrange("(m p) t -> p m t", p=P)
                if h == 1:
                    # first head-pair done: ship its half early
                    nc.sync.dma_start(aview[:, 0, :], ych[:, 0, :])
                if h == HL - 1:
                    # block's y complete: ship the rest and gather the 4
                    # cores' head-quads (AllGather wire runs ~1.7x the
                    # ReduceScatter path — no CCE reduction involved)
                    ych = ych_tiles.pop(blk)
                    nc.sync.dma_start(aview[:, 1, :], ych[:, 1, :])
                    nc.gpsimd.collective_compute(
                        "AllGather",
                        mybir.AluOpType.bypass,
                        replica_groups=RGROUPS,
                        ins=[ag_in[blk][:]],
                        outs=[ytf[blk][:]],
                    )

            def po_group(c):
                """Load gathered y and project this core's 256 output
                rows with full-C contraction; write out directly."""
                q0, qw, nkt = BLOCKS[c]
                # SEPARATE tiles per load piece: a single yf tile makes
                # every po matmul wait all its DMAs (tile-level dep) —
                # at the tail that chained the even-half matmuls to the
                # final AllGather they don't need
                if c == NBLK - 1:
                    # gathered halves: ytf_last[m] rows g*128+p are the
                    # global features g*256 + m*128 + p, i.e. ci = 2g+m
                    halves = []
                    for m in range(MT):
                        yh = sb.tile([P, GR, qw], BF, tag=f"yfl{m}",
                                     bufs=1, name=f"yfl{m}")
                        yv = ytf_last[m][:].rearrange("(a p) t -> p a t",
                                                      p=P)
                        nc.sync.dma_start(yh[:], yv)
                        halves.append(yh)

                    def rhs(ci):
                        return halves[ci % MT][:, ci // MT, :]
                    cis = [0, 2, 4, 6, 1, 3, 5, 7]
                else:
                    yview = ytf[c][:].rearrange("(a p) t -> p a t", p=P)
                    parts = []
                    for q in range(4):
                        yp = sb.tile([P, 2, qw], BF, tag=f"yf{q}",
                                     bufs=2, name=f"yf{q}")
                        nc.sync.dma_start(yp[:], yview[:, 2 * q:2 * q + 2])
                        parts.append(yp)

                    def rhs(ci):
                        return parts[ci // 2][:, ci % 2, :]
                    cis = list(range(NCH))
                ob = sb.tile([P, MT, qw], BF, tag="ob", bufs=2, name="ob")
                for rt in range(MT):
                    po = ps.tile([P, qw], F32, tag="st", bufs=3, name="po")
                    for k, ci in enumerate(cis):
                        nc.tensor.matmul(
                            po[:], wo_sb[:, ci, rt * P:(rt + 1) * P],
                            rhs(ci),
                            start=(k == 0), stop=(k == NCH - 1),
                        )
                    nc.vector.tensor_copy(ob[:, rt, :], po[:])
                    nc.gpsimd.dma_start(
                        out[rt * P:(rt + 1) * P, q0:q0 + qw], ob[:, rt, :]
                    )

            # ---------- job stream ----------
            # each job: (phase1, phase2) — phase2 runs LA jobs later
            jobs = []

            def add_filler(fn, *args):
                jobs.append((lambda a=args: fn(*a), None))

            pending_norms = []

            def emit_norms():
                while pending_norms and len(jobs) >= pending_norms[0][0]:
                    _, b_, h_ = pending_norms.pop(0)
                    add_filler(norm, b_, h_)

            def add_pair(blk, h, pr, last):
                def p1(a=(blk, h, pr)):
                    return scores(*a)

                def p2(pt, a=(blk, h, pr), last=last):
                    pv(*a, pt)
                    if last:
                        stage_den(a[0], a[1])
                jobs.append((p1, p2))
                if last:
                    # norm job legal once p2 (den copy) has executed:
                    # p1(i) runs before p2(i-LA), so i_norm > i_last+LA
                    pending_norms.append((len(jobs) + LA, blk, h))

            # chunk 0 projections (immediate: block 0 needs them; solo
            # 512-wide so they only gate on the fast x_head DMAs)
            for wsb, dst in ((wq_sb, qT_sb), (wk_sb, kT_sb)):
                for mt in range(MT):
                    add_filler(proj_one, 0, wsb, mt, dst)
            for t16 in range(4):
                add_filler(v_tile, t16)

            # blocks with fillers woven in
            for blk, (q0, qw, nkt) in enumerate(BLOCKS):
                npr = nkt // 2
                for h in range(HL):
                    for pr in range(npr):
                        emit_norms()
                        add_pair(blk, h, pr, pr == npr - 1)
                    emit_norms()
                    # weave fillers after each head
                    if blk == 0:
                        if h == 0:
                            for wsb, dst in ((wq_sb, qT_sb), (wk_sb, kT_sb)):
                                for mt in range(MT):
                                    add_filler(proj_one, 1, wsb, mt, dst)
                        elif h == 1:
                            for t16 in range(4, 8):
                                add_filler(v_tile, t16)
                        elif h == 2:
                            for t16 in range(8, 12):
                                add_filler(v_tile, t16)
                        else:
                            for t16 in range(12, 16):
                                add_filler(v_tile, t16)
                    elif blk == 1:
                        if h == 0:
                            for mt in range(MT):
                                add_filler(proj_pair, 1, wq_sb, mt, qT_sb)
                        elif h == 2:
                            for mt in range(MT):
                                add_filler(proj_pair, 1, wk_sb, mt, kT_sb)
                    elif blk > 1 and h == 1:
                        # po(blk-2): its AllGather fired early in block
                        # blk-1 and has had a full block to fly
                        add_filler(po_group, blk - 2)
            # flush the pipeline (phase2 lags by LA) before the last pos
            for _ in range(LA):
                add_filler(keepwarm, 1)
            while pending_norms:
                emit_norms()
                if pending_norms:
                    add_filler(keepwarm, 1)
            add_filler(po_group, NBLK - 2)
            # cover the last AllGather's flight time at warm clock
            add_filler(keepwarm, 10)
            add_filler(po_group, NBLK - 1)

            # ---------- software-pipelined emission ----------
            for blk, (q0, qw, nkt) in enumerate(BLOCKS):
                ych_tiles[blk] = sb.tile([P, MT, qw], BF, tag="ych",
                                         bufs=2, name="ych")

            pending = []
            for i in range(len(jobs) + LA):
                if i < len(jobs):
                    p1, p2 = jobs[i]
                    r = p1()
                    pending.append((p2, r))
                if i >= LA:
                    p2, r = pending[i - LA]
                    if p2 is not None:
                        p2(r)



    nc.finalize()
    return nc


# pv() needs yt allocated; allocate inside pv via yt_tiles guard
_GRAPH = None


def _get_graph():
    global _GRAPH
    if _GRAPH is None:
        _GRAPH = build_graph()
    return _GRAPH


def prepare_in_maps(x, Wq, Wk, Wv, Wo):
    x = np.asarray(x, np.float32)
    Wq = np.asarray(Wq, np.float32)
    Wk = np.asarray(Wk, np.float32)
    Wv = np.asarray(Wv, np.float32)
    Wo = np.asarray(Wo, np.float32)

    bf = ml_dtypes.bfloat16
    xTh = [np.ascontiguousarray(x[b].T).astype(bf) for b in range(B)]
    in_maps = []
    for r in range(N_CORES):
        b, g = r // GR, r % GR
        sl = slice(g * DL, (g + 1) * DL)
        # mt-major pack [p, w, mt, ci, c] so each 128-feature half is a
        # contiguous DMA
        wqkv = np.empty((P, 3, MT, NCH, P), np.float32)
        for w, W in enumerate((Wq, Wk, Wv)):
            wqkv[:, w] = W[sl].T.reshape(NCH, P, MT, P).transpose(1, 2, 0, 3)
        # wo lhsT: out rows = this core's 256 channels, contract full C
        wo_lhsT = np.ascontiguousarray(Wo[sl].T)  # [C, DL]
        woT = wo_lhsT.reshape(NCH, P, DL).transpose(1, 0, 2)  # [p, ci, DL]
        wall = np.concatenate(
            [wqkv.reshape(P, 3 * NCH * DL), woT.reshape(P, NCH * DL)],
            axis=1,
        )
        in_maps.append({
            "xT": xTh[b],
            "wall": np.ascontiguousarray(wall).astype(bf),
        })
    return in_maps


def assemble_output(results):
    outT = np.empty((B, C, TL), np.float32)
    for r in range(N_CORES):
        b, g = r // GR, r % GR
        outT[b, g * DL:(g + 1) * DL] = np.asarray(
            results[r]["out"], np.float32
        )
    return np.ascontiguousarray(outT.transpose(0, 2, 1))  # [B, T, C]


def kernel(x, Wq, Wk, Wv, Wo):
    nc = _get_graph()
    in_maps = prepare_in_maps(x, Wq, Wk, Wv, Wo)
    res = run_bass_kernel_spmd(nc, in_maps, core_ids=list(range(N_CORES)))
    return assemble_output(res.results)


# revision 76
# speedup vs baseline: 1.0200x; 1.0200x over previous
"""Causal multi-head attention block (B=2, T=2048, C=1024, H=16) on 8 TRN2
NeuronCores.

Sharding: 2D tensor parallel — core r owns batch b = r//4 and head quad
g = r%4 (heads 4g..4g+3, feature slice [256g, 256g+256)). Each core
projects q/k/v for its 256 features over its batch's 2048 tokens
(x^T replicated per batch) and runs causal attention for its 4 heads.
Per 512-token block, the 4 cores of a batch AllGather their y shards
(~0.75 MB wire/core per block at 40-60 GB/s — measured ~1.7x faster
than the ReduceScatter-of-partial-outputs dual, which drags through the
CCE reduction path), then each core computes its own 256 output rows
with a full-C contraction and writes them straight to `out`.

On-device everything is feature-major (transposed) so the TensorEngine
contraction axis sits on SBUF partitions and the softmax denominator
arrives via a ones-column appended to V:

  qT/kT [128, 2, 2048] = W_shard @ x^T           (2 feature tiles)
  v_sb[128tok, kt, h, 0:64] = x_tile^T @ Wv      (v built NATURALLY per
                                                  token tile — no
                                                  transpose pass)
  ST tile [128k, 512q] = kT[64h slice].T @ qT    (contract d=64)
  PT = exp(ST * 1/sqrt(d))                       (no max-subtraction:
                                                  logits ~N(0,1))
  causal mask: PT[diag 128-block] *= tri (upper-triangular 0/1 bf16,
      vector multiply) — cheaper than injecting -1e9 into PSUM via
      matmul, which cost an ident LDWEIGHTS + matmul per diagonal tile
  yT [65, 512] += [v | 1].T @ PT                 (row 64 = softmax denom)
  ych[*, 512]  = yT[0:64] * bcast(recip(yT[64]))
  out[128rt]  += WoT[ci, rt] @ ytf_gathered[ci]  (2 row tiles × 8 ci)

Performance structure (from perfetto/HAM analysis): the PE idle-activity
throttle (HAM) halves the clock for any window containing idle time — a
dense-matmul microbench holds 2.4 GHz while the v1 kernel averaged
~0.63x. So the whole kernel is ONE software-pipelined job stream:
score-pair jobs and filler GEMM jobs (projection chunks, O-projection
groups, v tiles) are interleaved, with each pair's PV matmuls deferred
LA jobs so the PE never waits on the exp of the pair it just scored,
and each head's normalization deferred further so neither the PE nor
gpsimd ever head-of-line blocks on the vector eviction chain (a gpsimd
<-> vector cross-wait stalled every engine for ~20 us). The last
512-token block is split into two 256-wide halves so the final
AllGather is half-size and the previous one overlaps attention.

Inputs are bf16 (host-side cast); accumulation is f32 in PSUM; the
output shard is written bf16 and upcast to f32 on the host.
"""

import numpy as np
import ml_dtypes

import concourse.bacc as bacc
import concourse.mybir as mybir
import concourse.tile as tile
from concourse.bass_utils import run_bass_kernel_spmd
from concourse.masks import make_identity

N_CORES = 8
B, T, C, H = 2, 2048, 1024, 16
D = 64                # head dim
GR = 4                # head-group cores per batch
HL = 4                # heads per core
DL = HL * D           # local feature dim = 256
MT = DL // 128        # feature tiles per core = 2
TL = T                # local tokens per core = its batch's 2048
P = 128
NCH = C // P          # 8 contraction chunks for q/k/v projections
QCH = 512             # q-chunk / token chunk
NQC = TL // QCH       # 4 chunks
NKT = TL // P         # 16 k-tiles
NRT = C // P          # 8 output row tiles for partial O-proj
SCALE = 1.0 / np.sqrt(D)
LA = 2                # job-stream lookahead (score pairs ahead of PV)

BF = mybir.dt.bfloat16
F32 = mybir.dt.float32
AF = mybir.ActivationFunctionType

RGROUPS = [[0, 1, 2, 3], [4, 5, 6, 7]]


def build_graph():
    nc = bacc.Bacc("TRN2", target_bir_lowering=False, debug=False)

    xT = nc.dram_tensor("xT", [C, TL], BF, kind="ExternalInput")
    # qkv shards [p, w, ci, m(256)] then woT [p, ci2, rt*128]
    WQKV = 3 * NCH * DL
    WO = MT * C
    wall = nc.dram_tensor("wall", [P, WQKV + WO], BF, kind="ExternalInput")
    out = nc.dram_tensor("out", [DL, TL], BF, kind="ExternalOutput")

    with tile.TileContext(nc) as tc:
        with (
            tc.tile_pool(name="sb", bufs=1) as sb,
            tc.tile_pool(name="ps", bufs=1, space="PSUM") as ps,
            tc.tile_pool(name="dram", bufs=1, space="DRAM") as dram,
        ):
            # ---- loads ----
            # separate tiles per weight and per x-chunk group so a
            # consumer only waits on ITS producer DMA, not all of them
            wq_full = sb.tile([P, NCH * DL], BF, name="wq_full")
            wk_full = sb.tile([P, NCH * DL], BF, name="wk_full")
            wv_full = sb.tile([P, NCH * DL], BF, name="wv_full")
            wo_full = sb.tile([P, MT * C], BF, name="wo_full")
            # x split: first 512 tokens per ci land fast, rest batched
            x_head = sb.tile([P, NCH, QCH], BF, name="x_head")
            x_tail = sb.tile([P, NCH, TL - QCH], BF, name="x_tail")

            # ident + wsrc FIRST: the warmup matmuls gate on them, and
            # make_identity runs on gpsimd — queued behind the DMA
            # issues it delayed the very first matmul to ~12us
            ident = sb.tile([P, P], BF, name="ident")
            make_identity(nc, ident)
            wsrc = sb.tile([P, QCH], BF, name="wsrc")
            nc.vector.memset(wsrc[:], 0.5)

            def keepwarm(n):
                for _ in range(n):
                    wdst = ps.tile([P, QCH], F32, tag="st", bufs=3,
                                   name="wdst")
                    nc.tensor.matmul(wdst[:], ident[:], wsrc[:],
                                     start=True, stop=True)

            keepwarm(24)

            ENGS3 = (nc.scalar, nc.sync, nc.gpsimd)
            # weights are packed mt-major host-side so each mt half is a
            # contiguous 256 KB — the first projection job then gates on
            # a half-DMA instead of the whole weight
            WH = NCH * DL // 2
            nc.sync.dma_start(wq_full[:, 0:WH], wall[:, 0:WH])
            for ci in (1, 4):
                ENGS3[ci % 3].dma_start(
                    x_head[:, ci, :], xT[ci * P:(ci + 1) * P, 0:QCH]
                )
            nc.sync.dma_start(wq_full[:, WH:], wall[:, WH:2 * WH])
            for ci in (0, 3, 6, 2, 5):
                ENGS3[ci % 3].dma_start(
                    x_head[:, ci, :], xT[ci * P:(ci + 1) * P, 0:QCH]
                )
            nc.scalar.dma_start(wk_full[:, 0:WH], wall[:, 2 * WH:3 * WH])
            nc.scalar.dma_start(wk_full[:, WH:], wall[:, 3 * WH:4 * WH])
            nc.gpsimd.dma_start(wv_full[:],
                                wall[:, 2 * NCH * DL:3 * NCH * DL])
            nc.sync.dma_start(x_head[:, 7, :], xT[7 * P:8 * P, 0:QCH])
            for ci in range(NCH):
                ENGS3[ci % 3].dma_start(
                    x_tail[:, ci, :], xT[ci * P:(ci + 1) * P, QCH:TL]
                )
            nc.scalar.dma_start(wo_full[:], wall[:, WQKV:])

            def xap(ci, t0, t1):
                # token-range view across the head/tail split tiles
                assert t0 >= QCH or t1 <= QCH
                if t1 <= QCH:
                    return x_head[:, ci, t0:t1]
                return x_tail[:, ci, t0 - QCH:t1 - QCH]

            # mt-major: [p, mt, ci, 128]
            wq_sb = wq_full[:].rearrange("p (m a c) -> p m a c", m=MT, a=NCH)
            wk_sb = wk_full[:].rearrange("p (m a c) -> p m a c", m=MT, a=NCH)
            wv_sb = wv_full[:].rearrange("p (m a c) -> p m a c", m=MT, a=NCH)
            wo_sb = wo_full[:].rearrange("p (a r) -> p a r", a=NCH)

            # upper-triangular (q >= k) 0/1 mask for diagonal blocks
            tri = sb.tile([P, P], BF, name="tri")
            nc.gpsimd.memset(tri[:], 1.0)
            nc.gpsimd.affine_select(
                out=tri[:], in_=tri[:],
                compare_op=mybir.AluOpType.is_ge,
                fill=0.0, base=0, channel_multiplier=-1, pattern=[[1, P]],
            )

            qT_sb = sb.tile([P, MT, TL], BF, name="qT_sb")
            kT_sb = sb.tile([P, MT, TL], BF, name="kT_sb")
            # v natural layout, per 128-token tile per head: [64 v | 1]
            v_sb = sb.tile([P, NKT, HL, D + 1], BF, name="v_sb")
            nc.gpsimd.memset(v_sb[:], 1.0)

            # blocks: (q0, qw, nkt). The last 512-token block is split
            # into two 256-wide halves so its first RS overlaps the
            # remaining attention and the tail RS is half-size.
            BLOCKS = [
                (0, QCH, 4), (QCH, QCH, 8), (2 * QCH, QCH, 12),
                (3 * QCH, QCH // 2, 14), (3 * QCH + QCH // 2, QCH // 2, 16),
            ]
            NBLK = len(BLOCKS)

            # the LAST block's gather is split per feature half so the
            # first half flies while heads 2-3 are still normalizing
            ag_in = [
                dram.tile([DL, qw], BF, name=f"ag_in{c}")
                for c, (q0, qw, nkt) in enumerate(BLOCKS[:-1])
            ]
            ytf = [
                dram.tile([C, qw], BF, name=f"ytf{c}")
                for c, (q0, qw, nkt) in enumerate(BLOCKS[:-1])
            ]
            QWL = BLOCKS[-1][1]
            ag_last = [dram.tile([P, QWL], BF, name=f"ag_last{m}")
                       for m in range(MT)]
            ytf_last = [dram.tile([GR * P, QWL], BF, name=f"ytf_last{m}")
                        for m in range(MT)]

            # ---------- job bodies ----------
            def proj_one(tch, wsb, mt, dst):
                # single 512-wide chunk (used early: evicts ASAP)
                t0 = tch * QCH
                pj = ps.tile([P, QCH], F32, tag="st", bufs=3, name="pj")
                for ci in range(NCH):
                    nc.tensor.matmul(
                        pj[:], wsb[:, mt, ci, :],
                        xap(ci, t0, t0 + QCH),
                        start=(ci == 0), stop=(ci == NCH - 1),
                    )
                nc.vector.tensor_copy(dst[:, mt, t0:t0 + QCH], pj[:])

            def proj_pair(pch, wsb, mt, dst):
                # two 512-wide accumulation groups, one 1024-wide evict
                t0 = pch * 2 * QCH
                pj = ps.tile([P, 2 * QCH], F32, tag="st", bufs=3, name="pj")
                for half in range(2):
                    h0 = t0 + half * QCH
                    for ci in range(NCH):
                        nc.tensor.matmul(
                            pj[:, half * QCH:(half + 1) * QCH],
                            wsb[:, mt, ci, :],
                            xap(ci, h0, h0 + QCH),
                            start=(ci == 0), stop=(ci == NCH - 1),
                        )
                nc.vector.tensor_copy(
                    dst[:, mt, t0:t0 + 2 * QCH], pj[:]
                )

            def v_tile(t16):
                # v_nat [128 tok, 256 feat] = x_tile^T @ Wv
                vps = ps.tile([P, DL], F32, tag="st", bufs=3, name="vps")
                for ci in range(NCH):
                    nc.tensor.matmul(
                        vps[:], xap(ci, t16 * P, (t16 + 1) * P),
                        wv_sb[:, :, ci, :],
                        start=(ci == 0), stop=(ci == NCH - 1),
                    )
                nc.vector.tensor_copy(
                    v_sb[:, t16, :, 0:D],
                    vps[:].rearrange("p (h x) -> p h x", h=HL),
                )

            ych_tiles = {}
            yt_tiles = {}

            def scores(blk, h, pr):
                """Scores + exp (+ causal mask) for one k-tile pair."""
                q0, qw, nkt = BLOCKS[blk]
                mt, hh = h // 2, h % 2
                rsl = slice(hh * D, (hh + 1) * D)
                st = ps.tile([P, 2 * qw], F32, tag="st", bufs=3, name="st")
                pt = sb.tile([P, 2 * qw], BF, tag="pt", bufs=5, name="pt")
                diag = []
                for half in range(2):
                    kt = 2 * pr + half
                    qv = max(kt * P - q0, 0)
                    nc.tensor.matmul(
                        st[:, half * qw + qv:(half + 1) * qw],
                        kT_sb[rsl, mt, kt * P:(kt + 1) * P],
                        qT_sb[rsl, mt, q0 + qv:q0 + qw],
                        start=True, stop=True,
                    )
                    if kt * P >= q0:
                        diag.append(half * qw + qv)
                qv0 = max(2 * pr * P - q0, 0)
                nc.scalar.activation(
                    pt[:, qv0:], st[:, qv0:], AF.Exp, scale=float(SCALE)
                )
                for c0 in diag:
                    nc.vector.tensor_mul(
                        pt[:, c0:c0 + P], pt[:, c0:c0 + P], tri[:]
                    )
                return pt

            def pv(blk, h, pr, pt):
                q0, qw, nkt = BLOCKS[blk]
                yt = yt_tiles.get((blk, h))
                if yt is None:
                    yt = yt_tiles[(blk, h)] = ps.tile(
                        [D + 1, qw], F32, tag="yt", bufs=2, name="yt"
                    )
                for half in range(2):
                    kt = 2 * pr + half
                    qv = max(kt * P - q0, 0)
                    nc.tensor.matmul(
                        yt[:, qv:qw],
                        v_sb[:, kt, h, :],
                        pt[:, half * qw + qv:(half + 1) * qw],
                        start=(kt == 0), stop=(kt == nkt - 1),
                    )

            den_tiles = {}

            def stage_den(blk, h):
                # with the last pv: denominator row PSUM -> SBUF bf16
                q0, qw, nkt = BLOCKS[blk]
                yt = yt_tiles[(blk, h)]
                den = sb.tile([1, qw], F32, tag="den", bufs=4, name="den")
                nc.vector.tensor_copy(den[:], yt[D:D + 1, :])
                den_tiles[(blk, h)] = den

            def norm(blk, h):
                """Scheduled >=LA+1 jobs after the head's last pair so
                the gpsimd broadcast never head-of-line blocks against
                the vector den copy (that cross-wait stalled ~20us)."""
                q0, qw, nkt = BLOCKS[blk]
                yt = yt_tiles.pop((blk, h))
                den = den_tiles.pop((blk, h))
                bc = sb.tile([D, qw], F32, tag="bc", bufs=3, name="bc")
                nc.gpsimd.partition_broadcast(bc[:], den[:])
                rcp = sb.tile([D, qw], F32, tag="rcp", bufs=3, name="rcp")
                scr = sb.tile([D, qw], F32, tag="scr", bufs=3, name="scr")
                nc.vector.reciprocal_approx_accurate(
                    rcp[:], bc[:], scratch=scr[:]
                )
                mt, hh = h // 2, h % 2
                ych = ych_tiles[blk]
                nc.vector.tensor_mul(
                    ych[hh * D:(hh + 1) * D, mt, :], yt[0:D, :], rcp[:]
                )
                if blk == NBLK - 1:
                    # tail block: gather each feature half independently
                    # (NOT per head: sub-128KB collectives pay ~5-10us
                    # of per-op stream overhead and serialize the tail)
                    if h == 1:
                        nc.sync.dma_start(ag_last[0][:], ych[:, 0, :])
                        nc.gpsimd.collective_compute(
                            "AllGather", mybir.AluOpType.bypass,
                            replica_groups=RGROUPS,
                            ins=[ag_last[0][:]], outs=[ytf_last[0][:]],
                        )
                    elif h == HL - 1:
                        ych = ych_tiles.pop(blk)
                        nc.sync.dma_start(ag_last[1][:], ych[:, 1, :])
                        nc.gpsimd.collective_compute(
                            "AllGather", mybir.AluOpType.bypass,
                            replica_groups=RGROUPS,
                            ins=[ag_last[1][:]], outs=[ytf_last[1][:]],
                        )
                    return
                aview = ag_in[blk][:].rearrange("(m p) t -> p m t", p=P)
                if h == 1:
                    # first head-pair done: ship its half early
                    nc.sync.dma_start(aview[:, 0, :], ych[:, 0, :])
                if h == HL - 1:
                    # block's y complete: ship the rest and gather the 4
                    # cores' head-quads (AllGather wire runs ~1.7x the
                    # ReduceScatter path — no CCE reduction involved)
                    ych = ych_tiles.pop(blk)
                    nc.sync.dma_start(aview[:, 1, :], ych[:, 1, :])
                    nc.gpsimd.collective_compute(
                        "AllGather",
                        mybir.AluOpType.bypass,
                        replica_groups=RGROUPS,
                        ins=[ag_in[blk][:]],
                        outs=[ytf[blk][:]],
                    )

            def po_group(c):
                """Load gathered y and project this core's 256 output
                rows with full-C contraction; write out directly."""
                q0, qw, nkt = BLOCKS[c]
                # SEPARATE tiles per load piece: a single yf tile makes
                # every po matmul wait all its DMAs (tile-level dep) —
                # at the tail that chained the even-half matmuls to the
                # final AllGather they don't need
                if c == NBLK - 1:
                    # gathered halves: ytf_last[m] rows g*128+p are the
                    # global features g*256 + m*128 + p, i.e. ci = 2g+m
                    halves = []
                    for m in range(MT):
                        yh = sb.tile([P, GR, qw], BF, tag=f"yfl{m}",
                                     bufs=1, name=f"yfl{m}")
                        yv = ytf_last[m][:].rearrange("(a p) t -> p a t",
                                                      p=P)
                        nc.sync.dma_start(yh[:], yv)
                        halves.append(yh)

                    def rhs(ci):
                        return halves[ci % MT][:, ci // MT, :]
                    cis = [0, 2, 4, 6, 1, 3, 5, 7]
                else:
                    yview = ytf[c][:].rearrange("(a p) t -> p a t", p=P)
                    parts = []
                    for q in range(4):
                        yp = sb.tile([P, 2, qw], BF, tag=f"yf{q}",
                                     bufs=2, name=f"yf{q}")
                        nc.sync.dma_start(yp[:], yview[:, 2 * q:2 * q + 2])
                        parts.append(yp)

                    def rhs(ci):
                        return parts[ci // 2][:, ci % 2, :]
                    cis = list(range(NCH))
                ob = sb.tile([P, MT, qw], BF, tag="ob", bufs=2, name="ob")
                for rt in range(MT):
                    po = ps.tile([P, qw], F32, tag="st", bufs=3, name="po")
                    for k, ci in enumerate(cis):
                        nc.tensor.matmul(
                            po[:], wo_sb[:, ci, rt * P:(rt + 1) * P],
                            rhs(ci),
                            start=(k == 0), stop=(k == NCH - 1),
                        )
                    nc.vector.tensor_copy(ob[:, rt, :], po[:])
                    nc.gpsimd.dma_start(
                        out[rt * P:(rt + 1) * P, q0:q0 + qw], ob[:, rt, :]
                    )

            # ---------- job stream ----------
            # each job: (phase1, phase2) — phase2 runs LA jobs later
            jobs = []

            def add_filler(fn, *args):
                jobs.append((lambda a=args: fn(*a), None))

            pending_norms = []

            def emit_norms():
                while pending_norms and len(jobs) >= pending_norms[0][0]:
                    _, b_, h_ = pending_norms.pop(0)
                    add_filler(norm, b_, h_)

            def add_pair(blk, h, pr, last):
                def p1(a=(blk, h, pr)):
                    return scores(*a)

                def p2(pt, a=(blk, h, pr), last=last):
                    pv(*a, pt)
                    if last:
                        stage_den(a[0], a[1])
                jobs.append((p1, p2))
                if last:
                    # norm job legal once p2 (den copy) has executed:
                    # p1(i) runs before p2(i-LA), so i_norm > i_last+LA
                    pending_norms.append((len(jobs) + LA, blk, h))

            # chunk 0 projections (immediate: block 0 needs them; solo
            # 512-wide so they only gate on the fast x_head DMAs)
            for wsb, dst in ((wq_sb, qT_sb), (wk_sb, kT_sb)):
                for mt in range(MT):
                    add_filler(proj_one, 0, wsb, mt, dst)
            for t16 in range(4):
                add_filler(v_tile, t16)

            # blocks with fillers woven in
            for blk, (q0, qw, nkt) in enumerate(BLOCKS):
                npr = nkt // 2
                for h in range(HL):
                    for pr in range(npr):
                        emit_norms()
                        add_pair(blk, h, pr, pr == npr - 1)
                    emit_norms()
                    # weave fillers after each head
                    if blk == 0:
                        if h == 0:
                            for wsb, dst in ((wq_sb, qT_sb), (wk_sb, kT_sb)):
                                for mt in range(MT):
                                    add_filler(proj_one, 1, wsb, mt, dst)
                        elif h == 1:
                            for t16 in range(4, 8):
                                add_filler(v_tile, t16)
                        elif h == 2:
                            for t16 in range(8, 12):
                                add_filler(v_tile, t16)
                        else:
                            for t16 in range(12, 16):
                                add_filler(v_tile, t16)
                    elif blk == 1:
                        if h == 0:
                            for mt in range(MT):
                                add_filler(proj_pair, 1, wq_sb, mt, qT_sb)
                        elif h == 2:
                            for mt in range(MT):
                                add_filler(proj_pair, 1, wk_sb, mt, kT_sb)
                    elif blk == 2 and h == 2:
                        # po(0) one head later than the others: the
                        # FIRST AllGather consistently runs 2-3x slower
                        # (cold stream) and stalled po(0) ~12us at h1
                        add_filler(po_group, 0)
                    elif blk > 2 and h == 1:
                        # po(blk-2): its AllGather fired early in block
                        # blk-1 and has had a full block to fly
                        add_filler(po_group, blk - 2)
            # flush the pipeline (phase2 lags by LA) before the last pos
            for _ in range(LA):
                add_filler(keepwarm, 1)
            while pending_norms:
                emit_norms()
                if pending_norms:
                    add_filler(keepwarm, 1)
            add_filler(po_group, NBLK - 2)
            # cover the last AllGather's flight time at warm clock
            add_filler(keepwarm, 10)
            add_filler(po_group, NBLK - 1)

            # ---------- software-pipelined emission ----------
            for blk, (q0, qw, nkt) in enumerate(BLOCKS):
                ych_tiles[blk] = sb.tile([P, MT, qw], BF, tag="ych",
                                         bufs=2, name="ych")

            pending = []
            for i in range(len(jobs) + LA):
                if i < len(jobs):
                    p1, p2 = jobs[i]
                    r = p1()
                    pending.append((p2, r))
                if i >= LA:
                    p2, r = pending[i - LA]
                    if p2 is not None:
                        p2(r)



    nc.finalize()
    return nc


# pv() needs yt allocated; allocate inside pv via yt_tiles guard
_GRAPH = None


def _get_graph():
    global _GRAPH
    if _GRAPH is None:
        _GRAPH = build_graph()
    return _GRAPH


def prepare_in_maps(x, Wq, Wk, Wv, Wo):
    x = np.asarray(x, np.float32)
    Wq = np.asarray(Wq, np.float32)
    Wk = np.asarray(Wk, np.float32)
    Wv = np.asarray(Wv, np.float32)
    Wo = np.asarray(Wo, np.float32)

    bf = ml_dtypes.bfloat16
    xTh = [np.ascontiguousarray(x[b].T).astype(bf) for b in range(B)]
    in_maps = []
    for r in range(N_CORES):
        b, g = r // GR, r % GR
        sl = slice(g * DL, (g + 1) * DL)
        # mt-major pack [p, w, mt, ci, c] so each 128-feature half is a
        # contiguous DMA
        wqkv = np.empty((P, 3, MT, NCH, P), np.float32)
        for w, W in enumerate((Wq, Wk, Wv)):
            wqkv[:, w] = W[sl].T.reshape(NCH, P, MT, P).transpose(1, 2, 0, 3)
        # wo lhsT: out rows = this core's 256 channels, contract full C
        wo_lhsT = np.ascontiguousarray(Wo[sl].T)  # [C, DL]
        woT = wo_lhsT.reshape(NCH, P, DL).transpose(1, 0, 2)  # [p, ci, DL]
        wall = np.concatenate(
            [wqkv.reshape(P, 3 * NCH * DL), woT.reshape(P, NCH * DL)],
            axis=1,
        )
        in_maps.append({
            "xT": xTh[b],
            "wall": np.ascontiguousarray(wall).astype(bf),
        })
    return in_maps


def assemble_output(results):
    outT = np.empty((B, C, TL), np.float32)
    for r in range(N_CORES):
        b, g = r // GR, r % GR
        outT[b, g * DL:(g + 1) * DL] = np.asarray(
            results[r]["out"], np.float32
        )
    return np.ascontiguousarray(outT.transpose(0, 2, 1))  # [B, T, C]


def kernel(x, Wq, Wk, Wv, Wo):
    nc = _get_graph()
    in_maps = prepare_in_maps(x, Wq, Wk, Wv, Wo)
    res = run_bass_kernel_spmd(nc, in_maps, core_ids=list(range(N_CORES)))
    return assemble_output(res.results)


# revision 79
# speedup vs baseline: 1.1058x; 1.0842x over previous
"""Causal multi-head attention block (B=2, T=2048, C=1024, H=16) on 8 TRN2
NeuronCores.

Sharding: 2D tensor parallel — core r owns batch b = r//4 and head quad
g = r%4 (heads 4g..4g+3, feature slice [256g, 256g+256)). Each core
projects q/k/v for its 256 features over its batch's 2048 tokens
(x^T replicated per batch) and runs causal attention for its 4 heads.
Per 512-token block, the 4 cores of a batch AllGather their y shards
(~0.75 MB wire/core per block at 40-60 GB/s — measured ~1.7x faster
than the ReduceScatter-of-partial-outputs dual, which drags through the
CCE reduction path), then each core computes its own 256 output rows
with a full-C contraction and writes them straight to `out`.

On-device everything is feature-major (transposed) so the TensorEngine
contraction axis sits on SBUF partitions and the softmax denominator
arrives via a ones-column appended to V:

  qT/kT [128, 2, 2048] = W_shard @ x^T           (2 feature tiles)
  v_sb[128tok, kt, h, 0:64] = x_tile^T @ Wv      (v built NATURALLY per
                                                  token tile — no
                                                  transpose pass)
  ST tile [128k, 512q] = kT[64h slice].T @ qT    (contract d=64)
  PT = exp(ST * 1/sqrt(d))                       (no max-subtraction:
                                                  logits ~N(0,1))
  causal mask: PT[diag 128-block] *= tri (upper-triangular 0/1 bf16,
      vector multiply) — cheaper than injecting -1e9 into PSUM via
      matmul, which cost an ident LDWEIGHTS + matmul per diagonal tile
  yT [65, 512] += [v | 1].T @ PT                 (row 64 = softmax denom)
  ych[*, 512]  = yT[0:64] * bcast(recip(yT[64]))
  out[128rt]  += WoT[ci, rt] @ ytf_gathered[ci]  (2 row tiles × 8 ci)

Performance structure (from perfetto/HAM analysis): the PE idle-activity
throttle (HAM) halves the clock for any window containing idle time — a
dense-matmul microbench holds 2.4 GHz while the v1 kernel averaged
~0.63x. So the whole kernel is ONE software-pipelined job stream:
score-pair jobs and filler GEMM jobs (projection chunks, O-projection
groups, v tiles) are interleaved, with each pair's PV matmuls deferred
LA jobs so the PE never waits on the exp of the pair it just scored,
and each head's normalization deferred further so neither the PE nor
gpsimd ever head-of-line blocks on the vector eviction chain (a gpsimd
<-> vector cross-wait stalled every engine for ~20 us). The last
512-token block is split into two 256-wide halves so the final
AllGather is half-size and the previous one overlaps attention.

Inputs are bf16 (host-side cast); accumulation is f32 in PSUM; the
output shard is written bf16 and upcast to f32 on the host.
"""

import numpy as np
import ml_dtypes

import concourse.bacc as bacc
import concourse.mybir as mybir
import concourse.tile as tile
from concourse.bass_utils import run_bass_kernel_spmd
from concourse.masks import make_identity

N_CORES = 8
B, T, C, H = 2, 2048, 1024, 16
D = 64                # head dim
GR = 4                # head-group cores per batch
HL = 4                # heads per core
DL = HL * D           # local feature dim = 256
MT = DL // 128        # feature tiles per core = 2
TL = T                # local tokens per core = its batch's 2048
P = 128
NCH = C // P          # 8 contraction chunks for q/k/v projections
QCH = 512             # q-chunk / token chunk
NQC = TL // QCH       # 4 chunks
NKT = TL // P         # 16 k-tiles
NRT = C // P          # 8 output row tiles for partial O-proj
SCALE = 1.0 / np.sqrt(D)
LA = 2                # job-stream lookahead (score pairs ahead of PV)

BF = mybir.dt.bfloat16
F32 = mybir.dt.float32
AF = mybir.ActivationFunctionType

RGROUPS = [[0, 1, 2, 3], [4, 5, 6, 7]]


def build_graph():
    nc = bacc.Bacc("TRN2", target_bir_lowering=False, debug=False)

    xT = nc.dram_tensor("xT", [C, TL], BF, kind="ExternalInput")
    # qkv shards [p, w, ci, m(256)] then woT [p, ci2, rt*128]
    WQKV = 3 * NCH * DL
    WO = MT * C
    wall = nc.dram_tensor("wall", [P, WQKV + WO], BF, kind="ExternalInput")
    out = nc.dram_tensor("out", [DL, TL], BF, kind="ExternalOutput")

    with tile.TileContext(nc) as tc:
        with (
            tc.tile_pool(name="sb", bufs=1) as sb,
            tc.tile_pool(name="ps", bufs=1, space="PSUM") as ps,
            tc.tile_pool(name="dram", bufs=1, space="DRAM") as dram,
        ):
            # ---- loads ----
            # separate tiles per weight and per x-chunk group so a
            # consumer only waits on ITS producer DMA, not all of them
            wq_full = sb.tile([P, NCH * DL], BF, name="wq_full")
            wk_full = sb.tile([P, NCH * DL], BF, name="wk_full")
            wv_full = sb.tile([P, NCH * DL], BF, name="wv_full")
            wo_full = sb.tile([P, MT * C], BF, name="wo_full")
            # x split: first 512 tokens per ci land fast, rest batched
            x_head = sb.tile([P, NCH, QCH], BF, name="x_head")
            x_tail = sb.tile([P, NCH, TL - QCH], BF, name="x_tail")

            # ident + wsrc FIRST: the warmup matmuls gate on them, and
            # make_identity runs on gpsimd — queued behind the DMA
            # issues it delayed the very first matmul to ~12us
            ident = sb.tile([P, P], BF, name="ident")
            make_identity(nc, ident)
            wsrc = sb.tile([P, QCH], BF, name="wsrc")
            nc.vector.memset(wsrc[:], 0.5)

            def keepwarm(n):
                for _ in range(n):
                    wdst = ps.tile([P, QCH], F32, tag="st", bufs=3,
                                   name="wdst")
                    nc.tensor.matmul(wdst[:], ident[:], wsrc[:],
                                     start=True, stop=True)

            keepwarm(24)

            ENGS3 = (nc.scalar, nc.sync, nc.gpsimd)
            # weights are packed mt-major host-side so each mt half is a
            # contiguous 256 KB — the first projection job then gates on
            # a half-DMA instead of the whole weight
            WH = NCH * DL // 2
            nc.sync.dma_start(wq_full[:, 0:WH], wall[:, 0:WH])
            for ci in (1, 4):
                ENGS3[ci % 3].dma_start(
                    x_head[:, ci, :], xT[ci * P:(ci + 1) * P, 0:QCH]
                )
            nc.sync.dma_start(wq_full[:, WH:], wall[:, WH:2 * WH])
            for ci in (0, 3, 6, 2, 5):
                ENGS3[ci % 3].dma_start(
                    x_head[:, ci, :], xT[ci * P:(ci + 1) * P, 0:QCH]
                )
            nc.scalar.dma_start(wk_full[:, 0:WH], wall[:, 2 * WH:3 * WH])
            nc.scalar.dma_start(wk_full[:, WH:], wall[:, 3 * WH:4 * WH])
            nc.gpsimd.dma_start(wv_full[:],
                                wall[:, 2 * NCH * DL:3 * NCH * DL])
            nc.sync.dma_start(x_head[:, 7, :], xT[7 * P:8 * P, 0:QCH])
            for ci in range(NCH):
                ENGS3[ci % 3].dma_start(
                    x_tail[:, ci, :], xT[ci * P:(ci + 1) * P, QCH:TL]
                )
            nc.scalar.dma_start(wo_full[:], wall[:, WQKV:])

            def xap(ci, t0, t1):
                # token-range view across the head/tail split tiles
                assert t0 >= QCH or t1 <= QCH
                if t1 <= QCH:
                    return x_head[:, ci, t0:t1]
                return x_tail[:, ci, t0 - QCH:t1 - QCH]

            # mt-major: [p, mt, ci, 128]
            wq_sb = wq_full[:].rearrange("p (m a c) -> p m a c", m=MT, a=NCH)
            wk_sb = wk_full[:].rearrange("p (m a c) -> p m a c", m=MT, a=NCH)
            wv_sb = wv_full[:].rearrange("p (m a c) -> p m a c", m=MT, a=NCH)
            wo_sb = wo_full[:].rearrange("p (a r) -> p a r", a=NCH)

            # upper-triangular (q >= k) 0/1 mask for diagonal blocks
            tri = sb.tile([P, P], BF, name="tri")
            nc.gpsimd.memset(tri[:], 1.0)
            nc.gpsimd.affine_select(
                out=tri[:], in_=tri[:],
                compare_op=mybir.AluOpType.is_ge,
                fill=0.0, base=0, channel_multiplier=-1, pattern=[[1, P]],
            )

            qT_sb = sb.tile([P, MT, TL], BF, name="qT_sb")
            kT_sb = sb.tile([P, MT, TL], BF, name="kT_sb")
            # v natural layout, per 128-token tile per head: [64 v | 1]
            v_sb = sb.tile([P, NKT, HL, D + 1], BF, name="v_sb")
            nc.gpsimd.memset(v_sb[:], 1.0)

            # blocks: (q0, qw, nkt). The last 512-token block is split
            # into two 256-wide halves so its first RS overlaps the
            # remaining attention and the tail RS is half-size.
            BLOCKS = [
                (0, QCH, 4), (QCH, QCH, 8), (2 * QCH, QCH, 12),
                (3 * QCH, QCH // 2, 14), (3 * QCH + QCH // 2, QCH // 2, 16),
            ]
            NBLK = len(BLOCKS)

            ag_in = [
                dram.tile([DL, qw], BF, name=f"ag_in{c}")
                for c, (q0, qw, nkt) in enumerate(BLOCKS)
            ]
            ytf = [
                dram.tile([C, qw], BF, name=f"ytf{c}")
                for c, (q0, qw, nkt) in enumerate(BLOCKS)
            ]

            # ---------- job bodies ----------
            def proj_one(tch, wsb, mt, dst):
                # single 512-wide chunk (used early: evicts ASAP)
                t0 = tch * QCH
                pj = ps.tile([P, QCH], F32, tag="st", bufs=3, name="pj")
                for ci in range(NCH):
                    nc.tensor.matmul(
                        pj[:], wsb[:, mt, ci, :],
                        xap(ci, t0, t0 + QCH),
                        start=(ci == 0), stop=(ci == NCH - 1),
                    )
                nc.vector.tensor_copy(dst[:, mt, t0:t0 + QCH], pj[:])

            def proj_pair(pch, wsb, mt, dst):
                # two 512-wide accumulation groups, one 1024-wide evict
                t0 = pch * 2 * QCH
                pj = ps.tile([P, 2 * QCH], F32, tag="st", bufs=3, name="pj")
                for half in range(2):
                    h0 = t0 + half * QCH
                    for ci in range(NCH):
                        nc.tensor.matmul(
                            pj[:, half * QCH:(half + 1) * QCH],
                            wsb[:, mt, ci, :],
                            xap(ci, h0, h0 + QCH),
                            start=(ci == 0), stop=(ci == NCH - 1),
                        )
                nc.vector.tensor_copy(
                    dst[:, mt, t0:t0 + 2 * QCH], pj[:]
                )

            def v_tile(t16):
                # v_nat [128 tok, 256 feat] = x_tile^T @ Wv
                vps = ps.tile([P, DL], F32, tag="st", bufs=3, name="vps")
                for ci in range(NCH):
                    nc.tensor.matmul(
                        vps[:], xap(ci, t16 * P, (t16 + 1) * P),
                        wv_sb[:, :, ci, :],
                        start=(ci == 0), stop=(ci == NCH - 1),
                    )
                nc.vector.tensor_copy(
                    v_sb[:, t16, :, 0:D],
                    vps[:].rearrange("p (h x) -> p h x", h=HL),
                )

            ych_tiles = {}
            yt_tiles = {}

            def scores(blk, h, pr):
                """Scores + exp (+ causal mask) for one k-tile pair."""
                q0, qw, nkt = BLOCKS[blk]
                mt, hh = h // 2, h % 2
                rsl = slice(hh * D, (hh + 1) * D)
                st = ps.tile([P, 2 * qw], F32, tag="st", bufs=3, name="st")
                pt = sb.tile([P, 2 * qw], BF, tag="pt", bufs=5, name="pt")
                diag = []
                for half in range(2):
                    kt = 2 * pr + half
                    qv = max(kt * P - q0, 0)
                    nc.tensor.matmul(
                        st[:, half * qw + qv:(half + 1) * qw],
                        kT_sb[rsl, mt, kt * P:(kt + 1) * P],
                        qT_sb[rsl, mt, q0 + qv:q0 + qw],
                        start=True, stop=True,
                    )
                    if kt * P >= q0:
                        diag.append(half * qw + qv)
                qv0 = max(2 * pr * P - q0, 0)
                nc.scalar.activation(
                    pt[:, qv0:], st[:, qv0:], AF.Exp, scale=float(SCALE)
                )
                for c0 in diag:
                    nc.vector.tensor_mul(
                        pt[:, c0:c0 + P], pt[:, c0:c0 + P], tri[:]
                    )
                return pt

            def pv(blk, h, pr, pt):
                q0, qw, nkt = BLOCKS[blk]
                yt = yt_tiles.get((blk, h))
                if yt is None:
                    yt = yt_tiles[(blk, h)] = ps.tile(
                        [D + 1, qw], F32, tag="yt", bufs=2, name="yt"
                    )
                for half in range(2):
                    kt = 2 * pr + half
                    qv = max(kt * P - q0, 0)
                    nc.tensor.matmul(
                        yt[:, qv:qw],
                        v_sb[:, kt, h, :],
                        pt[:, half * qw + qv:(half + 1) * qw],
                        start=(kt == 0), stop=(kt == nkt - 1),
                    )

            den_tiles = {}

            def stage_den(blk, h):
                # with the last pv: denominator row PSUM -> SBUF bf16
                q0, qw, nkt = BLOCKS[blk]
                yt = yt_tiles[(blk, h)]
                den = sb.tile([1, qw], F32, tag="den", bufs=4, name="den")
                nc.vector.tensor_copy(den[:], yt[D:D + 1, :])
                den_tiles[(blk, h)] = den

            def norm(blk, h):
                """Scheduled >=LA+1 jobs after the head's last pair so
                the gpsimd broadcast never head-of-line blocks against
                the vector den copy (that cross-wait stalled ~20us)."""
                q0, qw, nkt = BLOCKS[blk]
                yt = yt_tiles.pop((blk, h))
                den = den_tiles.pop((blk, h))
                bc = sb.tile([D, qw], F32, tag="bc", bufs=3, name="bc")
                nc.gpsimd.partition_broadcast(bc[:], den[:])
                rcp = sb.tile([D, qw], F32, tag="rcp", bufs=3, name="rcp")
                scr = sb.tile([D, qw], F32, tag="scr", bufs=3, name="scr")
                nc.vector.reciprocal_approx_accurate(
                    rcp[:], bc[:], scratch=scr[:]
                )
                mt, hh = h // 2, h % 2
                ych = ych_tiles[blk]
                nc.vector.tensor_mul(
                    ych[hh * D:(hh + 1) * D, mt, :], yt[0:D, :], rcp[:]
                )
                # (a two-half tail gather was tried: the second AG gets
                # serialized ~6us behind the first on the CC stream and
                # ends LATER than one merged gather)
                aview = ag_in[blk][:].rearrange("(m p) t -> p m t", p=P)
                if h == 1:
                    # first head-pair done: ship its half early
                    nc.sync.dma_start(aview[:, 0, :], ych[:, 0, :])
                if h == HL - 1:
                    # block's y complete: ship the rest and gather the 4
                    # cores' head-quads (AllGather wire runs ~1.7x the
                    # ReduceScatter path — no CCE reduction involved)
                    ych = ych_tiles.pop(blk)
                    nc.sync.dma_start(aview[:, 1, :], ych[:, 1, :])
                    nc.gpsimd.collective_compute(
                        "AllGather",
                        mybir.AluOpType.bypass,
                        replica_groups=RGROUPS,
                        ins=[ag_in[blk][:]],
                        outs=[ytf[blk][:]],
                    )

            def po_group(c):
                """Load gathered y and project this core's 256 output
                rows with full-C contraction; write out directly."""
                q0, qw, nkt = BLOCKS[c]
                # SEPARATE tiles per load piece: a single yf tile makes
                # every po matmul wait all its DMAs (tile-level dep) —
                # at the tail that chained the even-half matmuls to the
                # final AllGather they don't need
                yview = ytf[c][:].rearrange("(a p) t -> p a t", p=P)
                parts = []
                for q in range(4):
                    yp = sb.tile([P, 2, qw], BF, tag=f"yf{q}",
                                 bufs=2, name=f"yf{q}")
                    nc.sync.dma_start(yp[:], yview[:, 2 * q:2 * q + 2])
                    parts.append(yp)

                def rhs(ci):
                    return parts[ci // 2][:, ci % 2, :]
                cis = list(range(NCH))
                ob = sb.tile([P, MT, qw], BF, tag="ob", bufs=2, name="ob")
                for rt in range(MT):
                    po = ps.tile([P, qw], F32, tag="st", bufs=3, name="po")
                    for k, ci in enumerate(cis):
                        nc.tensor.matmul(
                            po[:], wo_sb[:, ci, rt * P:(rt + 1) * P],
                            rhs(ci),
                            start=(k == 0), stop=(k == NCH - 1),
                        )
                    nc.vector.tensor_copy(ob[:, rt, :], po[:])
                    nc.gpsimd.dma_start(
                        out[rt * P:(rt + 1) * P, q0:q0 + qw], ob[:, rt, :]
                    )

            # ---------- job stream ----------
            # each job: (phase1, phase2) — phase2 runs LA jobs later
            jobs = []

            def add_filler(fn, *args):
                jobs.append((lambda a=args: fn(*a), None))

            pending_norms = []

            def emit_norms():
                while pending_norms and len(jobs) >= pending_norms[0][0]:
                    _, b_, h_ = pending_norms.pop(0)
                    add_filler(norm, b_, h_)

            def add_pair(blk, h, pr, last):
                def p1(a=(blk, h, pr)):
                    return scores(*a)

                def p2(pt, a=(blk, h, pr), last=last):
                    pv(*a, pt)
                    if last:
                        stage_den(a[0], a[1])
                jobs.append((p1, p2))
                if last:
                    # norm job legal once p2 (den copy) has executed:
                    # p1(i) runs before p2(i-LA), so i_norm > i_last+LA
                    pending_norms.append((len(jobs) + LA, blk, h))

            # chunk 0 projections (immediate: block 0 needs them; solo
            # 512-wide so they only gate on the fast x_head DMAs)
            for wsb, dst in ((wq_sb, qT_sb), (wk_sb, kT_sb)):
                for mt in range(MT):
                    add_filler(proj_one, 0, wsb, mt, dst)
            for t16 in range(4):
                add_filler(v_tile, t16)

            # blocks with fillers woven in
            for blk, (q0, qw, nkt) in enumerate(BLOCKS):
                npr = nkt // 2
                for h in range(HL):
                    for pr in range(npr):
                        emit_norms()
                        add_pair(blk, h, pr, pr == npr - 1)
                    emit_norms()
                    # weave fillers after each head
                    if blk == 0:
                        if h == 0:
                            for wsb, dst in ((wq_sb, qT_sb), (wk_sb, kT_sb)):
                                for mt in range(MT):
                                    add_filler(proj_one, 1, wsb, mt, dst)
                        elif h == 1:
                            for t16 in range(4, 8):
                                add_filler(v_tile, t16)
                        elif h == 2:
                            for t16 in range(8, 12):
                                add_filler(v_tile, t16)
                        else:
                            for t16 in range(12, 16):
                                add_filler(v_tile, t16)
                    elif blk == 1:
                        if h == 0:
                            for mt in range(MT):
                                add_filler(proj_pair, 1, wq_sb, mt, qT_sb)
                        elif h == 2:
                            for mt in range(MT):
                                add_filler(proj_pair, 1, wk_sb, mt, kT_sb)
                    elif blk == 2 and h == 2:
                        # po(0) one head later than the others: the
                        # FIRST AllGather consistently runs 2-3x slower
                        # (cold stream) and stalled po(0) ~12us at h1
                        add_filler(po_group, 0)
                    elif blk > 2 and h == 1:
                        # po(blk-2): its AllGather fired early in block
                        # blk-1 and has had a full block to fly
                        add_filler(po_group, blk - 2)
            # flush the pipeline (phase2 lags by LA) before the last pos
            for _ in range(LA):
                add_filler(keepwarm, 1)
            while pending_norms:
                emit_norms()
                if pending_norms:
                    add_filler(keepwarm, 1)
            add_filler(po_group, NBLK - 2)
            # cover the last AllGather's flight time at warm clock
            add_filler(keepwarm, 10)
            add_filler(po_group, NBLK - 1)

            # ---------- software-pipelined emission ----------
            for blk, (q0, qw, nkt) in enumerate(BLOCKS):
                ych_tiles[blk] = sb.tile([P, MT, qw], BF, tag="ych",
                                         bufs=2, name="ych")

            pending = []
            for i in range(len(jobs) + LA):
                if i < len(jobs):
                    p1, p2 = jobs[i]
                    r = p1()
                    pending.append((p2, r))
                if i >= LA:
                    p2, r = pending[i - LA]
                    if p2 is not None:
                        p2(r)



    nc.finalize()
    return nc


# pv() needs yt allocated; allocate inside pv via yt_tiles guard
_GRAPH = None


def _get_graph():
    global _GRAPH
    if _GRAPH is None:
        _GRAPH = build_graph()
    return _GRAPH


def prepare_in_maps(x, Wq, Wk, Wv, Wo):
    x = np.asarray(x, np.float32)
    Wq = np.asarray(Wq, np.float32)
    Wk = np.asarray(Wk, np.float32)
    Wv = np.asarray(Wv, np.float32)
    Wo = np.asarray(Wo, np.float32)

    bf = ml_dtypes.bfloat16
    xTh = [np.ascontiguousarray(x[b].T).astype(bf) for b in range(B)]
    in_maps = []
    for r in range(N_CORES):
        b, g = r // GR, r % GR
        sl = slice(g * DL, (g + 1) * DL)
        # mt-major pack [p, w, mt, ci, c] so each 128-feature half is a
        # contiguous DMA
        wqkv = np.empty((P, 3, MT, NCH, P), np.float32)
        for w, W in enumerate((Wq, Wk, Wv)):
            wqkv[:, w] = W[sl].T.reshape(NCH, P, MT, P).transpose(1, 2, 0, 3)
        # wo lhsT: out rows = this core's 256 channels, contract full C
        wo_lhsT = np.ascontiguousarray(Wo[sl].T)  # [C, DL]
        woT = wo_lhsT.reshape(NCH, P, DL).transpose(1, 0, 2)  # [p, ci, DL]
        wall = np.concatenate(
            [wqkv.reshape(P, 3 * NCH * DL), woT.reshape(P, NCH * DL)],
            axis=1,
        )
        in_maps.append({
            "xT": xTh[b],
            "wall": np.ascontiguousarray(wall).astype(bf),
        })
    return in_maps


def assemble_output(results):
    outT = np.empty((B, C, TL), np.float32)
    for r in range(N_CORES):
        b, g = r // GR, r % GR
        outT[b, g * DL:(g + 1) * DL] = np.asarray(
            results[r]["out"], np.float32
        )
    return np.ascontiguousarray(outT.transpose(0, 2, 1))  # [B, T, C]


def kernel(x, Wq, Wk, Wv, Wo):
    nc = _get_graph()
    in_maps = prepare_in_maps(x, Wq, Wk, Wv, Wo)
    res = run_bass_kernel_spmd(nc, in_maps, core_ids=list(range(N_CORES)))
    return assemble_output(res.results)
